# revision 27
# baseline (speedup 1.0000x reference)
"""MultiPropMLP (MoE-routed tiny MLP) Trainium2 kernel — expert-routed version.

Problem: out[n] = MLP_{idx[n]}(xs[n]) for N = 8192*128 samples, K = 8 experts,
MLP = 16 -> 64 -> relu -> 64 -> relu -> 1 with per-expert weights.

Sharding: expert-parallel over the 8 NeuronCores. The host groups samples by
expert (np.argsort on idx — this IS the sharding step for an MoE) and core c
receives expert c's bucket, padded to a fixed capacity of NU*1024 samples.
Each core then runs a pure dense 16->64->64->1 MLP chain on its samples with
its single expert's weights: no masking, no select, no index upload, and 8x
less matmul+evac volume than the dense all-K formulation. The host scatters
the per-core results back through the inverse permutation (data movement
only; all FLOPs happen on device).

Per-core layout: samples are split into two halves A/B that ride the PE
partition dim together via block-diagonal weights, so one matmul with a
512-column moving tensor processes 1024 samples:

  unit u (1024 samples = half-A cols [512u,512u+512) + half-B same cols):
    L0: h0[128,512] = blockdiag(W0,W0).T @ xT[32,512]          (psum, 1 bank)
    ev: h0_sb = relu(h0 + b0)                                  (ACT or DVE)
    L1: h1[128,512] = blockdiag(W1,W1).T @ h0_sb   (into half of 2-bank pair)
    ev: h1_sb = relu(h1 + b1), one [128,1024] evac per 2 units
    L2: accumulate unit's two scalars into rows (2j,2j+1) of a shared
        [16,512] psum tile (j = u%8) via a W2 stack that is zero outside
        those rows — PE cost is free-dim only, so this is as cheap as any
        other packing and needs no nonzero partition bases.
  per 8 units: copy the [16,512] l2 bank to sbuf staging
  per 8 blocks: one DMA staging [16, 4096] -> HBM (plain contiguous slice)

xT arrives from the host already feature-major ([32, half]: rows 0-15 =
features of half-A, 16-31 = half-B), so the device does zero transposes.
b2 (a scalar per expert) is added on the host after download.

The issue order is software-pipelined: iteration `it` issues
L0(it), L1(it-3), L2(it-6), so the in-order PE queue never stalls on a PSUM
evacuation. Only ACT and DVE can read PSUM on TRN2 (GPSIMD cannot), so
evacuations are greedily load-balanced across those two using the cost
model's per-op times. PSUM budget: h0 2 banks + h1 2x2-bank pairs + l2 2
banks = 8 banks exactly. A dummy 1-column Relu at kernel start preloads the
ACT activation table while the first xT chunk is still in flight; the first
chunks are small so compute starts early.

PE floor: 3 matmuls x 512 moving columns per 1024 samples = 1.5 cycles per
sample (~83us/core for 129 units at 2.4GHz); the evac engines run just
below that, so the kernel is PE-bound.

Note: walrus in this toolchain accepts only ONE sync-wait per instruction;
_split_ctrl_waits() hoists Tile's multi-waits onto single-wait nops.
"""

import numpy as np

R, S, D_IN, WIDTH, K = 8192, 128, 16, 64, 8
N = R * S
NCORES = 8
P = 128
GROUP = 512            # samples per half-group = matmul moving columns
UNIT = 2 * GROUP       # samples per unit (2 halves packed on partitions)
S1, S2 = 5, 10         # software-pipeline staggers for L1 / L2
BLK = 16               # units accumulated per l2 psum bank ([32, 512])
BATCH = 8              # l2 blocks per staging buffer / output DMA
# measured cost-model evac times (ns): [128,512] and [128,1024] on ACT / DVE
EV1_COST = (612.0, 658.0)
EV2_COST = (1038.0, 1192.0)

_cache = {}
_dbg_sched = {}  # engine -> [label], in issue order (for trace analysis only)


def _chunk_plan(nu):
    """Input-DMA chunks as (start_unit, n_units): small first chunks so
    compute starts early, then steady 8-unit chunks."""
    sizes = []
    for s in (4,):
        if sum(sizes) < nu:
            sizes.append(min(s, nu - sum(sizes)))
    while sum(sizes) < nu:
        sizes.append(min(8, nu - sum(sizes)))
    starts = np.concatenate([[0], np.cumsum(sizes)[:-1]]).astype(int)
    return list(zip(starts.tolist(), sizes))


def _build_nc(nu):
    import concourse.bass as bass
    import concourse.mybir as mybir
    from concourse import tile

    f32 = mybir.dt.float32
    f32r = mybir.dt.float32r
    half = nu * GROUP
    nblk = -(-nu // BLK)
    nc = bass.Bass()

    L2W = 2 * BLK * BLK  # W2 stack: BLK variants of [P, 2*BLK]
    WPK = P + L2W + P + 2  # l1w | l2w stack | l0w(rows 0:32) | b0 | b1
    xt_c = nc.dram_tensor("xt_c", [32, half], f32r, kind="ExternalInput")
    wpk = nc.dram_tensor("wpk", [P, WPK], f32r, kind="ExternalInput")
    out_c = nc.dram_tensor("out_c", [2 * BLK, nblk * GROUP], f32, kind="ExternalOutput")

    relu = mybir.ActivationFunctionType.Relu
    add = mybir.AluOpType.add
    mx = mybir.AluOpType.max

    chunks = _chunk_plan(nu)
    unit_chunk = np.zeros(nu, int)
    for ci, (st, n) in enumerate(chunks):
        unit_chunk[st : st + n] = ci

    _dbg_sched.clear()
    _dbg_sched.update({"PE": [], "ACT": [], "DVE": [], "SP": []})

    with tile.TileContext(nc) as tc:
        with (
            tc.tile_pool(name="const", bufs=1) as cpool,
            tc.tile_pool(name="xt", bufs=4) as xpool,
            tc.tile_pool(name="h0", bufs=7) as h0pool,
            tc.tile_pool(name="h1", bufs=4) as h1pool,
            tc.tile_pool(name="stg", bufs=2) as spool,
            tc.tile_pool(name="ps_h0", bufs=3, space="PSUM") as ps_h0,
            tc.tile_pool(name="ps_h1", bufs=2, space="PSUM") as ps_h1,
            tc.tile_pool(name="ps_l2", bufs=1, space="PSUM") as ps_l2,
        ):
            wpk_sb = cpool.tile([P, WPK], f32r, tag="wpk")
            nc.sync.dma_start(wpk_sb[:], wpk[:])
            l1w_sb = wpk_sb[:, 0:P]
            l2w_sb = wpk_sb[:, P : P + L2W]
            l0w_sb = wpk_sb[0:32, P + L2W : P + L2W + P]
            b0_sb = wpk_sb[:, WPK - 2 : WPK - 1].bitcast(f32)
            b1_sb = wpk_sb[:, WPK - 1 : WPK].bitcast(f32)

            # preload the Relu activation table while the first DMA runs
            warm = cpool.tile([P, 1], f32, tag="warm")
            nc.vector.memset(warm[:], 0.0)
            nc.scalar.activation(warm[:], warm[:], relu)

            # greedy ACT/DVE load balancing by accumulated busy time
            busy = [0.0, 0.0]

            def pick(costs):
                e = 0 if busy[0] + costs[0] <= busy[1] + costs[1] else 1
                busy[e] += costs[e]
                return e

            def ev_relu(costs, o, i, b, lbl=""):
                if pick(costs) == 0:
                    _dbg_sched["ACT"].append(lbl)
                    nc.scalar.activation(o, i, relu, bias=b)
                else:
                    _dbg_sched["DVE"].append(lbl)
                    nc.vector.tensor_scalar(o, i, b, 0.0, add, mx)

            def ev_copy(costs, o, i, lbl=""):
                if pick(costs) == 0:
                    _dbg_sched["ACT"].append(lbl)
                    nc.scalar.copy(o, i)
                else:
                    _dbg_sched["DVE"].append(lbl)
                    nc.vector.tensor_copy(o, i)

            xt_tiles = {}
            next_chunk = [0]

            def ensure_chunks(unit):
                target = unit_chunk[min(unit, nu - 1)]
                while next_chunk[0] <= target:
                    ci = next_chunk[0]
                    st, n = chunks[ci]
                    t = xpool.tile([32, 8 * GROUP], f32r, tag="xt")
                    nc.sync.dma_start(
                        t[:, 0 : n * GROUP],
                        xt_c[:, st * GROUP : (st + n) * GROUP],
                    )
                    xt_tiles[ci] = (t, st)
                    next_chunk[0] += 1

            h0_sb = {}
            h1_pair = {}
            l2_tiles = {}
            stg = {}

            for it in range(nu + S2):
                if it < nu:
                    u = it
                    ensure_chunks(min(u + 16, nu - 1))
                    ci = unit_chunk[u]
                    t, st = xt_tiles[ci]
                    ps = ps_h0.tile([P, GROUP], f32, tag="h0ps")
                    _dbg_sched["PE"].append(f"L0({u})")
                    nc.tensor.matmul(
                        ps[:], l0w_sb,
                        t[:, (u - st) * GROUP : (u - st + 1) * GROUP],
                        start=True, stop=True,
                    )
                    sb = h0pool.tile([P, GROUP], f32r, tag="h0sb")
                    ev_relu(EV1_COST, sb[:], ps[:], b0_sb, f"ev0({u})")
                    h0_sb[u] = sb
                u = it - S1
                if 0 <= u < nu:
                    pi, hf = divmod(u, 2)
                    if hf == 0:
                        h1_pair[pi] = (
                            ps_h1.tile([P, 2 * GROUP], f32, tag="h1ps",
                                       name="h1ps"),
                            h1pool.tile([P, 2 * GROUP], f32r, tag="h1sb",
                                        name="h1sb"),
                        )
                    ps, sb = h1_pair[pi]
                    _dbg_sched["PE"].append(f"L1({u})")
                    nc.tensor.matmul(
                        ps[:, hf * GROUP : (hf + 1) * GROUP], l1w_sb,
                        h0_sb.pop(u)[:], start=True, stop=True,
                    )
                    if hf == 1 or u == nu - 1:
                        w = (hf + 1) * GROUP
                        ev_relu(
                            EV2_COST if hf == 1 else EV1_COST,
                            sb[:, 0:w], ps[:, 0:w], b1_sb, f"ev1({u})",
                        )
                u = it - S2
                if 0 <= u < nu:
                    b, j = divmod(u, BLK)
                    if j == 0:
                        l2_tiles[b] = ps_l2.tile([2 * BLK, GROUP], f32, tag="l2",
                                                 name="l2ps")
                    last = u == nu - 1
                    _dbg_sched["PE"].append(f"L2({u})")
                    nc.tensor.matmul(
                        l2_tiles[b][:],
                        l2w_sb[:, 2 * BLK * j : 2 * BLK * (j + 1)],
                        h1_pair[u // 2][1][:, (u % 2) * GROUP : (u % 2 + 1) * GROUP],
                        start=(j == 0), stop=(j == BLK - 1 or last),
                    )
                    if u % 2 == 1 or last:
                        h1_pair.pop(u // 2)
                    if j == BLK - 1 or last:
                        s, t_in = divmod(b, BATCH)
                        if t_in == 0:
                            stg["tile"] = spool.tile(
                                [2 * BLK, BATCH * GROUP], f32, tag="stg", name="stg"
                            )
                            stg["s"] = s
                        lt = l2_tiles.pop(b)
                        ev_copy(
                            EV1_COST,
                            stg["tile"][:, t_in * GROUP : (t_in + 1) * GROUP],
                            lt[:], f"stg({b})",
                        )
                        if t_in == BATCH - 1 or last:
                            w = (t_in + 1) * GROUP
                            o = stg["s"] * BATCH * GROUP
                            nc.sync.dma_start(
                                out_c[:, o : o + w], stg["tile"][:, 0:w]
                            )

    _split_ctrl_waits(nc, mybir)
    return nc


def _split_ctrl_waits(nc, mybir):
    """walrus in this container accepts only one sync-wait per instruction;
    Tile attaches one wait per dependency lane. Hoist extras onto preceding
    single-wait nops on the same engine (equivalent ordering semantics)."""
    for bb in nc.main_func.blocks:
        newlist = []
        changed = False
        for ins in bb.instructions:
            si = ins.sync_info
            if si is not None and len(si.on_wait) > 1:
                waits = list(si.on_wait)
                for j, w in enumerate(waits[:-1]):
                    nop = mybir.InstNoOp(name=f"{ins.name}-wsplit-{j}", ins=[], outs=[])
                    nop.engine = ins.engine
                    nop.sync_info = mybir.SyncInfo(on_wait=[w], on_update=[])
                    newlist.append(nop)
                si.on_wait = [waits[-1]]
                ins.sync_info = si
                changed = True
            newlist.append(ins)
        if changed:
            bb.instructions = newlist
    return nc


def kernel(idxs, xs, W0, b0, W1, b1, W2, b2):
    from concourse.bass_utils import run_bass_kernel_spmd

    idx = np.asarray(idxs).reshape(-1)
    xs_flat = np.ascontiguousarray(np.asarray(xs, np.float32).reshape(N, D_IN))
    W0 = np.asarray(W0, np.float32)
    b0 = np.asarray(b0, np.float32)
    W1 = np.asarray(W1, np.float32)
    b1 = np.asarray(b1, np.float32)
    W2 = np.asarray(W2, np.float32)
    b2 = np.asarray(b2, np.float32)

    counts = np.bincount(idx, minlength=K)
    order = np.argsort(idx, kind="stable")
    bounds = np.concatenate([[0], np.cumsum(counts)])

    nu = max(S2 + 2, -(-int(counts.max()) // UNIT))
    if nu not in _cache:
        _cache[nu] = _build_nc(nu)
    nc = _cache[nu]
    cap = nu * UNIT
    half = nu * GROUP
    nblk = -(-nu // BLK)

    xs_sorted = xs_flat[order]
    in_maps = []
    for c in range(NCORES):
        n_c = int(counts[c])
        pad = np.zeros((cap, D_IN), np.float32)
        pad[:n_c] = xs_sorted[bounds[c] : bounds[c + 1]]
        xt = np.empty((32, half), np.float32)
        xt[0:16] = pad[:half].T
        xt[16:32] = pad[half:].T
        l2w = 2 * BLK * BLK
        wpk = np.zeros((P, P + l2w + P + 2), np.float32)
        wpk[0:64, 0:64] = W1[c]
        wpk[64:128, 64:128] = W1[c]
        for j in range(BLK):
            wpk[0:64, P + 2 * BLK * j + 2 * j] = W2[c, :, 0]
            wpk[64:128, P + 2 * BLK * j + 2 * j + 1] = W2[c, :, 0]
        wpk[0:16, P + l2w : P + l2w + 64] = W0[c]
        wpk[16:32, P + l2w + 64 : P + l2w + 128] = W0[c]
        wpk[0:64, P + l2w + P] = b0[c]
        wpk[64:128, P + l2w + P] = b0[c]
        wpk[0:64, P + l2w + P + 1] = b1[c]
        wpk[64:128, P + l2w + P + 1] = b1[c]
        in_maps.append(dict(xt_c=np.ascontiguousarray(xt), wpk=wpk))

    res = run_bass_kernel_spmd(nc, in_maps, list(range(NCORES))).results

    out = np.empty(N, np.float32)
    for c in range(NCORES):
        oc = np.asarray(res[c]["out_c"], np.float32).reshape(BLK, 2, nblk, GROUP)
        o_sorted = np.empty(cap, np.float32)
        for h in range(2):
            # sample h*half + 512*(BLK*b + j) + col  ==  oc[j, h, b, col]
            o_sorted[h * half : (h + 1) * half] = np.transpose(
                oc[:, h], (1, 0, 2)
            ).reshape(-1)[: half]
        n_c = int(counts[c])
        out[order[bounds[c] : bounds[c + 1]]] = o_sorted[:n_c] + b2[c, 0]
    return out.reshape(R, S, 1)
